# revision 1
# baseline (speedup 1.0000x reference)
"""BitLinear (BitNet b1.58) forward kernel for Trainium2, 8 NeuronCores.

Computes  y = einsum('bsi,oi->bso', x, w_ste) + bias  where
  scale  = max(mean(|W|), 1e-8)
  w_q    = clip(round(W/scale), -1, 1)   (ternary, realized as a threshold:
           w_q = (w > scale/2) - (w < -scale/2), exactly equivalent under
           round-half-to-even)
  w_ste  = w_q * scale  (forward value)

Sharding: data-parallel over rows. Each core owns 2048 rows of x
(= one batch element) and the full weight. On device each core:
  phase A: streams W once to compute scale (abs-sum reduce + cross-partition
           all-reduce), while x loads into SBUF (resident, bf16)
  phase B: per 256-wide out-feature chunk: stream W f32, ternary-quantize to
           bf16, then PE matmul (K=4096 accumulated in PSUM f32), apply
           scale + bias on the way out.

x is staged pre-transposed [in_f, rows] in bf16 (matmul needs the
contraction dim on partitions for both operands; W is staged transposed
[in_f, out_f] in f32 so quantization happens on device at full precision).
"""

import numpy as np
import ml_dtypes

import concourse.tile as tile
import concourse.mybir as mybir
from concourse import bacc, bass_isa
from concourse.bass import ts
from concourse.bass_utils import run_bass_kernel_spmd

N_CORES = 8
IN_F = 4096
OUT_F = 4096
ROWS_PER_CORE = 2048
P = 128                   # SBUF partitions
KT = IN_F // P            # 32 k-tiles along contraction
MT = ROWS_PER_CORE // P   # 16 row-tiles per core
OCH = 256                 # out-feature chunk = matmul free dim
NCH = OUT_F // OCH        # 16 chunks
QS = 8                    # k-tiles per quantize slab

F32 = mybir.dt.float32
BF16 = mybir.dt.bfloat16

LAST_RESULTS = None
_NC_CACHE = {}


def _build():
    nc = bacc.Bacc(
        "TRN2", target_bir_lowering=False, debug=False, num_devices=N_CORES
    )
    xt = nc.dram_tensor(
        "xt", [IN_F, ROWS_PER_CORE], BF16, kind="ExternalInput"
    ).ap()
    wt = nc.dram_tensor("wt", [IN_F, OUT_F], F32, kind="ExternalInput").ap()
    bias = nc.dram_tensor("bias", [1, OUT_F], F32, kind="ExternalInput").ap()
    y = nc.dram_tensor(
        "y", [ROWS_PER_CORE, OUT_F], F32, kind="ExternalOutput"
    ).ap()

    with tile.TileContext(nc) as tc:
        with (
            tc.tile_pool(name="xp", bufs=1) as xp,
            tc.tile_pool(name="redp", bufs=1) as redp,
            tc.tile_pool(name="psum", bufs=4, space="PSUM") as pp,
        ):
            # x resident in SBUF: [128, 32 k-tiles, 2048 rows] bf16
            xsb = xp.tile([P, KT, ROWS_PER_CORE], BF16)
            xt_r = xt.rearrange("(kt p) r -> p kt r", p=P)
            for i in range(KT):
                nc.sync.dma_start(out=xsb[:, i, :], in_=xt_r[:, i, :])

            # ---- phase A: scale = max(mean(|W|), 1e-8) ----
            partials = redp.tile([P, KT], F32)
            wt_r = wt.rearrange("(kt p) c -> p kt c", p=P)
            with tc.tile_pool(name="sw", bufs=3) as swp:
                for i in range(KT):
                    stile = swp.tile([P, OUT_F], F32)
                    nc.sync.dma_start(out=stile, in_=wt_r[:, i, :])
                    nc.vector.tensor_reduce(
                        out=partials[:, i : i + 1],
                        in_=stile,
                        axis=mybir.AxisListType.X,
                        op=mybir.AluOpType.add,
                        apply_absolute_value=True,
                    )
            acc = redp.tile([P, 1], F32)
            nc.vector.tensor_reduce(
                out=acc,
                in_=partials,
                axis=mybir.AxisListType.X,
                op=mybir.AluOpType.add,
            )
            allsum = redp.tile([P, 1], F32)
            nc.gpsimd.partition_all_reduce(
                allsum, acc, channels=P, reduce_op=bass_isa.ReduceOp.add
            )
            scale_bc = redp.tile([P, 1], F32)
            nc.vector.tensor_scalar(
                out=scale_bc,
                in0=allsum,
                scalar1=1.0 / float(IN_F * OUT_F),
                scalar2=1e-8,
                op0=mybir.AluOpType.mult,
                op1=mybir.AluOpType.max,
            )
            tpos = redp.tile([P, 1], F32)
            tneg = redp.tile([P, 1], F32)
            nc.vector.tensor_scalar_mul(tpos, scale_bc, 0.5)
            nc.vector.tensor_scalar_mul(tneg, scale_bc, -0.5)

            # ---- phase B: quantize + matmul per out-feature chunk ----
            with (
                tc.tile_pool(name="wf", bufs=2) as wfp,
                tc.tile_pool(name="neg", bufs=2) as negp,
                tc.tile_pool(name="wq", bufs=2) as wqp,
                tc.tile_pool(name="bt", bufs=2) as btp,
                tc.tile_pool(name="yp", bufs=4) as yp,
            ):
                for j in range(NCH):
                    jo = j * OCH
                    wq = wqp.tile([P, KT, OCH], BF16)
                    for s in range(KT // QS):
                        wf = wfp.tile([P, QS, OCH], F32)
                        for q in range(QS):
                            i = s * QS + q
                            nc.sync.dma_start(
                                out=wf[:, q, :],
                                in_=wt[i * P : (i + 1) * P, jo : jo + OCH],
                            )
                        negt = negp.tile([P, QS, OCH], BF16)
                        wq_slab = wq[:, s * QS : (s + 1) * QS, :]
                        nc.vector.tensor_scalar(
                            out=wq_slab,
                            in0=wf,
                            scalar1=tpos,
                            scalar2=None,
                            op0=mybir.AluOpType.is_gt,
                        )
                        nc.vector.tensor_scalar(
                            out=negt,
                            in0=wf,
                            scalar1=tneg,
                            scalar2=None,
                            op0=mybir.AluOpType.is_lt,
                        )
                        nc.vector.tensor_sub(wq_slab, wq_slab, negt)

                    bt = btp.tile([P, OCH], F32)
                    nc.sync.dma_start(
                        out=bt, in_=bias[0:1, jo : jo + OCH].broadcast_to([P, OCH])
                    )
                    for m in range(MT):
                        ps = pp.tile([P, OCH], F32)
                        for i in range(KT):
                            nc.tensor.matmul(
                                ps,
                                xsb[:, i, ts(m, P)],
                                wq[:, i, :],
                                start=(i == 0),
                                stop=(i == KT - 1),
                            )
                        ysb = yp.tile([P, OCH], F32)
                        nc.vector.tensor_scalar(
                            out=ysb,
                            in0=ps,
                            scalar1=scale_bc,
                            scalar2=None,
                            op0=mybir.AluOpType.mult,
                        )
                        nc.vector.tensor_add(ysb, ysb, bt)
                        nc.sync.dma_start(
                            out=y[ts(m, P), jo : jo + OCH], in_=ysb
                        )

    nc.compile()
    return nc


def _get_nc():
    if "nc" not in _NC_CACHE:
        _NC_CACHE["nc"] = _build()
    return _NC_CACHE["nc"]


def kernel(x, weight, bias):
    global LAST_RESULTS
    x = np.asarray(x)
    weight = np.asarray(weight, dtype=np.float32)
    bias = np.asarray(bias, dtype=np.float32)
    b, s, _ = x.shape
    rows = b * s
    assert rows == N_CORES * ROWS_PER_CORE

    xf = np.ascontiguousarray(x.reshape(rows, IN_F).astype(np.float32))
    wt = np.ascontiguousarray(weight.T)  # [in_f, out_f] f32
    b2 = np.ascontiguousarray(bias.reshape(1, OUT_F))

    in_maps = []
    for c in range(N_CORES):
        xs = xf[c * ROWS_PER_CORE : (c + 1) * ROWS_PER_CORE]
        xtc = np.ascontiguousarray(xs.astype(ml_dtypes.bfloat16).T)
        in_maps.append({"xt": xtc, "wt": wt, "bias": b2})

    nc = _get_nc()
    res = run_bass_kernel_spmd(nc, in_maps, core_ids=list(range(N_CORES)))
    LAST_RESULTS = res
    y = np.concatenate(
        [res.results[c]["y"] for c in range(N_CORES)], axis=0
    )
    return np.ascontiguousarray(y.reshape(b, s, OUT_F).astype(np.float32))
